# revision 45
# baseline (speedup 1.0000x reference)
"""Trainium2 Bass kernel for a 2-layer GCN link-prediction model (DDI-style graph).

Math refactor (vs the PyG-style reference):
  gcn(h,W,b)[d] = dis[d] * (sum_{e: dst=d, incl self-loop} (dis[src_e] * h[src_e])) @ W + b
with dis = deg^{-1/2}. All stationary weights are folded on the host:
  - L1 table: embpW = dis * (emb @ W1)  (bf16), so layer 1 is
      U1[d] = sum_e embpW[src_e];  g' = dis * relu(dis*U1 + b1)
    (b1 enters the PSUM accumulation as a rank-1 matmul sqrt(deg)_d x b1).
  - L2 output: v2 = dis * U2 where U2[d] = sum_e g'[src_e]; W2/b2/dW1/db1 are
    folded into the decode: E1 = W2@dW1_top, E2 = W2@dW1_bot,
    bb = b2@dW1_top + b2@dW1_bot + db1, so
      hdec = relu(v2[a]@E1 + v2[b]@E2 + bb);  logits = hdec@dW2 + db2.
Each layer: gather table rows by src (4 SWDGE queues; random 256B rows are
engine-latency-bound) -> 0/1-indicator matmul (segmented sum by dst on the PE)
-> per-tile scale epilogue. Edge-parallel across 8 NeuronCores by dst-tile
ranges; tables are exchanged with chunked AllGathers laid out chunk-major with
a chunk boundary at row LO so lo/hi gathers of the next phase start as soon as
their chunks land. Gather streams are sorted by src row for HBM locality.
"""

import sys
import numpy as np
import ml_dtypes

sys.path.insert(0, "/opt/trn_rl_repo")

import concourse.bass as bass
import concourse.bacc as bacc
import concourse.mybir as mybir
import concourse.tile as tile
from concourse import bass_utils

BF16 = ml_dtypes.bfloat16

N_NODES = 50000
N_EDGES = 800000
N_QUERY = 200000
H = 128          # embed == hidden
NCLS = 86
P = 128
NCORES = 8
TPC = 49                 # dst tiles per core
NT = TPC * NCORES        # 392 global tiles (incl 1 pad tile)
NPAD = NT * P            # 50176
LO = 32768               # int16 gather index split
GROUP = 2                # conv slots per gather group
QSL = 512                # decode queries per slice
LB = 4                   # decode slices batched per logits DMA
NQUEUE = 4               # SWDGE queues (ucode max)
AGB = [0, 11, 22, 32, 41, 49]   # AG chunk bounds (slots); 32 -> row 32768 == LO
AGCH = len(AGB) - 1

TRACE = False            # set True (e.g. from test.py) to capture an NTFF profile
RUN_KWARGS = {}
LAST_EXEC_NS = None
LAST_RESULTS = None


def _wrap_idx(idx_list):
    """Wrap an index list (len % 128 == 0, int16) into the dma_gather SBUF
    layout: element j at [j % 16, j // 16], replicated across the 8 groups of
    16 partitions. Returns [128, len/16] int16."""
    L = len(idx_list)
    assert L % 128 == 0
    base = np.asarray(idx_list, np.int16).reshape(L // 16, 16).T  # [16, L/16]
    return np.tile(base, (8, 1))


def _ceil_div(a, b):
    return -(-a // b)


def _row_perm():
    """Table rows are laid out chunk-major (AG chunk, core, slot) so that each
    chunked AllGather writes one contiguous region. Returns newpos[t] for
    global tile t = c*TPC + j."""
    newpos = np.empty(NT, np.int64)
    for k in range(AGCH):
        w = AGB[k + 1] - AGB[k]
        for c in range(NCORES):
            for j in range(AGB[k], AGB[k + 1]):
                newpos[c * TPC + j] = NCORES * AGB[k] + c * w + (j - AGB[k])
    return newpos


def _prep_conv(edge_index, rowof):
    """Sort edges (plus self-loops) by dst, shard by dst-tile ranges, split by
    src row < LO for int16 gather indices. Within each group, slot streams are
    packed at ROW granularity with shared (max-over-core) offsets: boundary
    chunks serve two slots via separate indicator columns, so padding is only
    the cross-core variance, not 128-rounding per slot."""
    src = rowof(np.asarray(edge_index[0], np.int64))
    dst = np.asarray(edge_index[1], np.int64)

    deg = np.bincount(dst, minlength=NPAD).astype(np.float32) + 1.0

    order = np.argsort(dst, kind="stable")
    ssrc = src[order]
    sdst = dst[order]
    ptr = np.searchsorted(sdst, np.arange(0, NT * P + 1, P))

    lo_src = [[None] * TPC for _ in range(NCORES)]
    lo_dl = [[None] * TPC for _ in range(NCORES)]
    hi_src = [[None] * TPC for _ in range(NCORES)]
    hi_dl = [[None] * TPC for _ in range(NCORES)]
    for c in range(NCORES):
        for j in range(TPC):
            t = c * TPC + j
            e0, e1 = ptr[t], ptr[t + 1]
            es = ssrc[e0:e1]
            dl = (sdst[e0:e1] - t * P).astype(np.int64)
            m = es < LO
            lo_src[c][j] = es[m]
            lo_dl[c][j] = dl[m]
            hi_src[c][j] = es[~m] - LO
            hi_dl[c][j] = dl[~m]

    # shared per-slot row counts (max over cores)
    R_lo = [max(len(lo_src[c][j]) for c in range(NCORES)) for j in range(TPC)]
    R_hi = [max(len(hi_src[c][j]) for c in range(NCORES)) for j in range(TPC)]

    groups = []
    ch_total = 0     # gather chunks so far (gb columns)
    col_total = 0    # dstloc columns so far (indicator views)
    lo_cols = 0      # idx slab columns (int16, 16-wrapped)
    hi_cols = 0
    for g0 in range(0, TPC, GROUP):
        js = list(range(g0, min(g0 + GROUP, TPC)))
        g = {"slots": js, "chunk_off": ch_total, "dstloc_off": col_total,
             "lo_idx_col": lo_cols, "hi_idx_col": hi_cols,
             "row_off_lo": {}, "row_off_hi": {}, "slot_chunks": {}}
        # lo stream: row offsets per slot, then chunk views
        off = 0
        for j in js:
            g["row_off_lo"][j] = off
            off += R_lo[j]
        n_lo_rows = off
        nlo = _ceil_div(n_lo_rows, P)
        off = 0
        for j in js:
            g["row_off_hi"][j] = off
            off += R_hi[j]
        n_hi_rows = off
        nhi = _ceil_div(n_hi_rows, P)
        g["n_lo_chunks"] = nlo
        g["n_chunks"] = nlo + nhi
        g["n_lo_rows"] = n_lo_rows
        g["n_hi_rows"] = n_hi_rows
        # assign (chunk, dloc col) views per slot
        ncols = 0
        for j in js:
            views = []
            for base, R, off_d, choff in ((g["row_off_lo"], R_lo, 0, 0),
                                          (g["row_off_hi"], R_hi, 0, nlo)):
                o = base[j]
                Rj = R[j]
                if Rj == 0:
                    continue
                c0, c1 = o // P, (o + Rj - 1) // P
                for ch in range(c0, c1 + 1):
                    views.append((choff + ch, ncols))
                    ncols += 1
            g["slot_chunks"][j] = views
        g["n_cols"] = ncols
        ch_total += g["n_chunks"]
        col_total += ncols
        lo_cols += nlo * 8
        hi_cols += nhi * 8
        groups.append(g)

    sched = {
        "groups": groups,
        "ch_total": ch_total,
        "col_total": col_total,
        "lo_cols": lo_cols,
        "hi_cols": hi_cols,
        "max_chunks": max(g["n_chunks"] for g in groups),
        "max_cols": max(g["n_cols"] for g in groups),
    }

    # per-core data arrays
    per_core = []
    for c in range(NCORES):
        idx_lo = np.zeros((P, lo_cols), np.int16)
        idx_hi = np.zeros((P, hi_cols), np.int16)
        dstloc = np.full((P, col_total), 255.0, BF16)
        for g in groups:
            nlo = g["n_lo_chunks"]
            lo_stream = np.zeros(nlo * P, np.int16)
            hi_stream = np.zeros((g["n_chunks"] - nlo) * P, np.int16)
            dl_lo = np.full(nlo * P, 255.0, np.float32)
            dl_hi = np.full((g["n_chunks"] - nlo) * P, 255.0, np.float32)
            for j in g["slots"]:
                o = g["row_off_lo"][j]
                n = len(lo_src[c][j])
                lo_stream[o : o + n] = lo_src[c][j].astype(np.int16)
                dl_lo[o : o + n] = lo_dl[c][j]
                o = g["row_off_hi"][j]
                n = len(hi_src[c][j])
                hi_stream[o : o + n] = hi_src[c][j].astype(np.int16)
                dl_hi[o : o + n] = hi_dl[c][j]
            # indicator views: rows of other slots masked to 255
            for j in g["slots"]:
                for (chg, colg) in g["slot_chunks"][j]:
                    col = g["dstloc_off"] + colg
                    if chg < nlo:
                        o, Rj, dlv, ch = g["row_off_lo"][j], len(lo_src[c][j]), dl_lo, chg
                    else:
                        o, Rj, dlv, ch = g["row_off_hi"][j], len(hi_src[c][j]), dl_hi, chg - nlo
                    colvals = np.full(P, 255.0, np.float32)
                    r0, r1 = ch * P, (ch + 1) * P
                    s0, s1 = max(r0, o), min(r1, o + Rj)
                    if s1 > s0:
                        colvals[s0 - r0 : s1 - r0] = dlv[s0:s1]
                    dstloc[:, col] = colvals.astype(BF16)
            if nlo:
                idx_lo[:, g["lo_idx_col"] : g["lo_idx_col"] + nlo * 8] = _wrap_idx(lo_stream)
            if g["n_chunks"] - nlo:
                idx_hi[:, g["hi_idx_col"] : g["hi_idx_col"] + (g["n_chunks"] - nlo) * 8] = _wrap_idx(hi_stream)
        per_core.append({"idx_lo": idx_lo, "idx_hi": idx_hi, "dstloc": dstloc})

    return sched, per_core, deg


def _prep_decode(edge_label_index, rowof):
    """Shard queries across cores, sort each core's queries into 4 groups by
    (a < LO, b < LO) with group order both-lo first (so the both-lo slices can
    start before the hi AG chunks land), sort within group by a row, pad each
    group to a global (max-over-core) multiple of QSL. Returns (schedule,
    per-core idx arrays, per-core permutation)."""
    a = rowof(np.asarray(edge_label_index[0], np.int64))
    b = rowof(np.asarray(edge_label_index[1], np.int64))
    qpc = N_QUERY // NCORES
    core_groups = []
    for c in range(NCORES):
        aa = a[c * qpc : (c + 1) * qpc]
        bb = b[c * qpc : (c + 1) * qpc]
        key = (aa >= LO) * 2 + (bb >= LO)
        gidx = []
        for k in range(4):
            ids = np.nonzero(key == k)[0]
            ids = ids[np.argsort(aa[ids], kind="stable")]
            gidx.append(ids)
        core_groups.append((aa, bb, gidx))
    G = [max(_ceil_div(len(core_groups[c][2][k]), QSL) for c in range(NCORES)) for k in range(4)]
    QS = sum(G)
    qpad = QS * QSL

    per_core = []
    perms = []
    for c in range(NCORES):
        aa, bb, gidx = core_groups[c]
        qa = np.zeros((P, QS * (QSL // 16)), np.int16)
        qb = np.zeros((P, QS * (QSL // 16)), np.int16)
        perm = np.full(qpad, -1, np.int64)
        col = 0
        pos = 0
        for k in range(4):
            ids = gidx[k]
            L = G[k] * QSL
            av = np.zeros(L, np.int64)
            bv = np.zeros(L, np.int64)
            av[: len(ids)] = aa[ids]
            bv[: len(ids)] = bb[ids]
            if k >= 2:
                av -= LO
                av[len(ids):] = 0
            if k % 2 == 1:
                bv -= LO
                bv[len(ids):] = 0
            perm[pos : pos + len(ids)] = c * qpc + ids
            for s in range(G[k]):
                qa[:, col : col + QSL // 16] = _wrap_idx(av[s * QSL : (s + 1) * QSL])
                qb[:, col : col + QSL // 16] = _wrap_idx(bv[s * QSL : (s + 1) * QSL])
                col += QSL // 16
            pos += L
        per_core.append({"qa": qa, "qb": qb})
        perms.append(perm)
    dec_sched = {"G": G, "QS": QS, "QPAD": qpad}
    return dec_sched, per_core, perms


def _build(sched, dec, lo_cols, hi_cols):
    """Build the 8-core SPMD Bass program."""
    nc = bacc.Bacc("TRN2", target_bir_lowering=False, debug=False, num_devices=NCORES,
                   num_swdge_queues=NQUEUE)
    f32, bf16, i16 = mybir.dt.float32, mybir.dt.bfloat16, mybir.dt.int16
    AF = mybir.ActivationFunctionType
    ALU = mybir.AluOpType

    groups = sched["groups"]
    QS, QPAD, G = dec["QS"], dec["QPAD"], dec["G"]

    # ---- I/O ----
    # embp: host-precomputed dis*(emb@W1) table in bf16, permuted rows
    embp_in = nc.dram_tensor("embp", [NPAD, H], bf16, kind="ExternalInput").ap()
    diss_in = nc.dram_tensor("dis_s", [P, TPC], f32, kind="ExternalInput").ap()
    sdeg_in = nc.dram_tensor("sdeg_row", [1, TPC * P], f32, kind="ExternalInput").ap()
    b1_in = nc.dram_tensor("b1", [1, H], f32, kind="ExternalInput").ap()
    e1_in = nc.dram_tensor("e1", [H, H], f32, kind="ExternalInput").ap()
    e2_in = nc.dram_tensor("e2", [H, H], f32, kind="ExternalInput").ap()
    bb_in = nc.dram_tensor("bbias", [H, 1], f32, kind="ExternalInput").ap()
    dw2_in = nc.dram_tensor("dw2", [H, NCLS], f32, kind="ExternalInput").ap()
    db2_in = nc.dram_tensor("db2", [NCLS, 1], f32, kind="ExternalInput").ap()
    ixlo_in = nc.dram_tensor("idx_lo", [P, lo_cols], i16, kind="ExternalInput").ap()
    ixhi_in = nc.dram_tensor("idx_hi", [P, hi_cols], i16, kind="ExternalInput").ap()
    dloc_in = nc.dram_tensor("dstloc", [P, sched["col_total"]], bf16, kind="ExternalInput").ap()
    qa_in = nc.dram_tensor("qa", [P, QS * (QSL // 16)], i16, kind="ExternalInput").ap()
    qb_in = nc.dram_tensor("qb", [P, QS * (QSL // 16)], i16, kind="ExternalInput").ap()
    selfrows_in = nc.dram_tensor("self_rows", [TPC * P, H], bf16, kind="ExternalInput").ap()
    logits_out = nc.dram_tensor("logitsT", [NCLS, QPAD], f32, kind="ExternalOutput").ap()

    # ---- internal DRAM ----
    g_shard = nc.dram_tensor("g_shard", [TPC * P, H], bf16).ap()
    g_tab = nc.dram_tensor("g_tab", [NPAD, H], bf16, addr_space="Shared").ap()
    v_shard = nc.dram_tensor("v_shard", [TPC * P, H], bf16).ap()
    v_tab = nc.dram_tensor("v_tab", [NPAD, H], bf16, addr_space="Shared").ap()

    # ---- constants ----
    ident_np = np.eye(P, dtype=BF16)
    iota_np = np.tile(np.arange(P, dtype=BF16)[None, :], (P, 1))
    ident_c = nc.inline_tensor(ident_np, "ident_c").ap()
    iota_c = nc.inline_tensor(iota_np, "iota_c").ap()

    MAXCH = sched["max_chunks"]
    MAXCOL = sched["max_cols"]
    rg = [list(range(NCORES))]

    with tile.TileContext(nc, trace_sim=False) as tc:
        import contextlib
        ctx = contextlib.ExitStack()
        with ctx:
            cpool = ctx.enter_context(tc.tile_pool(name="consts", bufs=1))
            gpool = ctx.enter_context(tc.tile_pool(name="gather", bufs=6))
            ipool = ctx.enter_context(tc.tile_pool(name="indic", bufs=3))
            spool = ctx.enter_context(tc.tile_pool(name="small", bufs=3))
            qpool = ctx.enter_context(tc.tile_pool(name="dec", bufs=4))
            lpool = ctx.enter_context(tc.tile_pool(name="lout", bufs=2))
            pp_u = ctx.enter_context(tc.tile_pool(name="ps_u", bufs=2, space="PSUM"))
            pp_d = ctx.enter_context(tc.tile_pool(name="ps_d", bufs=2, space="PSUM"))
            pp_t = ctx.enter_context(tc.tile_pool(name="ps_t", bufs=1, space="PSUM"))

            # ---------- constants / weights ----------
            ident = cpool.tile([P, P], bf16, tag="ident")
            nc.sync.dma_start(ident[:], ident_c[:])
            iota = cpool.tile([P, P], bf16, tag="iota")
            nc.sync.dma_start(iota[:], iota_c[:])

            def load_bf(ap_in, shape, tag):
                tf = cpool.tile(shape, f32, tag=tag + "_f")
                nc.sync.dma_start(tf[:], ap_in[:])
                tb = cpool.tile(shape, bf16, tag=tag)
                nc.vector.tensor_copy(tb[:], tf[:])
                return tb

            b1r = load_bf(b1_in, [1, H], "b1r")
            e1 = load_bf(e1_in, [H, H], "e1")
            e2 = load_bf(e2_in, [H, H], "e2")
            dw2 = load_bf(dw2_in, [H, NCLS], "dw2")
            sdegr = load_bf(sdeg_in, [1, TPC * P], "sdegr")
            bb = cpool.tile([H, 1], f32, tag="bb")
            nc.sync.dma_start(bb[:], bb_in[:])
            db2 = cpool.tile([NCLS, 1], f32, tag="db2")
            nc.sync.dma_start(db2[:], db2_in[:])

            dis_sh = cpool.tile([P, TPC], f32, tag="dis_sh")
            nc.sync.dma_start(dis_sh[:], diss_in[:])

            # conv edge streams -> SBUF (resident, reused by both layers)
            ixlo = cpool.tile([P, lo_cols], i16, tag="ixlo")
            nc.sync.dma_start(ixlo[:], ixlo_in[:])
            ixhi = cpool.tile([P, hi_cols], i16, tag="ixhi")
            nc.sync.dma_start(ixhi[:], ixhi_in[:])
            dloc = cpool.tile([P, sched["col_total"]], bf16, tag="dloc")
            nc.sync.dma_start(dloc[:], dloc_in[:])
            qa_sb = cpool.tile([P, QS * (QSL // 16)], i16, tag="qa")
            nc.sync.dma_start(qa_sb[:], qa_in[:])
            qb_sb = cpool.tile([P, QS * (QSL // 16)], i16, tag="qb")
            nc.sync.dma_start(qb_sb[:], qb_in[:])

            # L1 self rows: this core's shard of embpW (= dis*emb@W1)
            selfb1 = cpool.tile([P, TPC * H], bf16, tag="selfb1")
            nc.sync.dma_start(
                selfb1[:].rearrange("p (j e) -> p j e", e=H),
                selfrows_in[:].rearrange("(j p) e -> p j e", p=P),
            )
            selfb2 = cpool.tile([P, TPC * H], bf16, tag="selfb2")

            # ---------- conv layer ----------
            def conv_layer(tab, out_shard, out_tab, is_first):
                tab_lo = tab[0:LO, :]
                tab_hi = tab[LO:NPAD, :]
                next_bound = 1  # AG chunk k covers slots [AGB[k], AGB[k+1])
                for gi, g in enumerate(groups):
                    nch = g["n_chunks"]
                    nlo = g["n_lo_chunks"]
                    nhi = nch - nlo
                    q_lo = gi % NQUEUE
                    q_hi = (gi + NQUEUE // 2) % NQUEUE
                    gb = gpool.tile([P, MAXCH * H], bf16, tag="gb")
                    if nlo:
                        nc.gpsimd.dma_gather(
                            out_ap=gb[:, : nlo * H].rearrange("p (c e) -> p c e", e=H),
                            in_ap=tab_lo,
                            idxs_ap=ixlo[:, g["lo_idx_col"] : g["lo_idx_col"] + nlo * 8],
                            num_idxs=nlo * P,
                            num_idxs_reg=nlo * P,
                            elem_size=H,
                            single_packet=False,
                            queue_num=q_lo,
                        )
                    if nhi:
                        nc.gpsimd.dma_gather(
                            out_ap=gb[:, nlo * H : nch * H].rearrange("p (c e) -> p c e", e=H),
                            in_ap=tab_hi,
                            idxs_ap=ixhi[:, g["hi_idx_col"] : g["hi_idx_col"] + nhi * 8],
                            num_idxs=nhi * P,
                            num_idxs_reg=nhi * P,
                            elem_size=H,
                            single_packet=False,
                            queue_num=q_hi,
                        )
                    # indicator views for the whole group in one DVE op
                    ncol = g["n_cols"]
                    ind = ipool.tile([P, MAXCOL * H], bf16, tag="ind")
                    dl = dloc[:, g["dstloc_off"] : g["dstloc_off"] + ncol]
                    nc.vector.tensor_tensor(
                        ind[:, : ncol * H].rearrange("p (c e) -> p c e", e=H),
                        iota[:].unsqueeze(1).to_broadcast([P, ncol, H]),
                        dl.unsqueeze(2).to_broadcast([P, ncol, H]),
                        op=ALU.is_equal,
                    )
                    for j in g["slots"]:
                        chunks = g["slot_chunks"][j]
                        srow_ap = (selfb1 if is_first else selfb2)[:, j * H : (j + 1) * H]
                        u_ps = pp_u.tile([P, H], f32, tag="u")
                        for si, (ch, col) in enumerate(chunks):
                            nc.tensor.matmul(
                                out=u_ps[:],
                                lhsT=ind[:, col * H : (col + 1) * H],
                                rhs=gb[:, ch * H : (ch + 1) * H],
                                start=(si == 0),
                                stop=False,
                            )
                        if is_first:
                            # bias enters pre-scale as sqrt(deg)_d x b1 (rank-1)
                            nc.tensor.matmul(
                                out=u_ps[:],
                                lhsT=sdegr[:, j * P : (j + 1) * P],
                                rhs=b1r[:],
                                start=(len(chunks) == 0),
                                stop=False,
                            )
                        nc.tensor.matmul(
                            out=u_ps[:],
                            lhsT=ident[:],
                            rhs=srow_ap,
                            start=(len(chunks) == 0 and not is_first),
                            stop=True,
                        )
                        if is_first:
                            # g' = dis * relu(dis*U + b1): two per-partition scales
                            t_sb = spool.tile([P, H], bf16, tag="t_sb")
                            nc.scalar.activation(t_sb[:], u_ps[:], AF.Relu,
                                                 scale=dis_sh[:, j : j + 1])
                            ob = selfb2[:, j * H : (j + 1) * H]
                            nc.scalar.mul(ob, t_sb[:], mul=dis_sh[:, j : j + 1])
                        else:
                            # v2 = dis * U2
                            o_bf = spool.tile([P, H], bf16, tag="o_bf")
                            ob = o_bf[:]
                            nc.scalar.mul(ob, u_ps[:], mul=dis_sh[:, j : j + 1])
                        nc.sync.dma_start(out_shard[j * P : (j + 1) * P, :], ob)
                        # chunked AllGather: fire chunk k once its slots done
                        while next_bound <= AGCH and j + 1 >= AGB[next_bound]:
                            j0, j1 = AGB[next_bound - 1], AGB[next_bound]
                            nc.gpsimd.collective_compute(
                                "AllGather", mybir.AluOpType.bypass,
                                ins=[out_shard[j0 * P : j1 * P, :]],
                                outs=[out_tab[NCORES * j0 * P : NCORES * j1 * P, :]],
                                replica_groups=rg,
                            )
                            next_bound += 1

            conv_layer(embp_in, g_shard, g_tab, True)
            conv_layer(g_tab, v_shard, v_tab, False)

            # ---------- decode ----------
            v_lo = v_tab[0:LO, :]
            v_hi = v_tab[LO:NPAD, :]
            sl = 0
            for k in range(4):
                a_tab = v_lo if k < 2 else v_hi
                b_tab = v_lo if k % 2 == 0 else v_hi
                for s in range(G[k]):
                    col = sl * (QSL // 16)
                    q_a = sl % NQUEUE
                    q_b = (sl + NQUEUE // 2) % NQUEUE
                    # non-transpose gathers (transpose-mode gathers are
                    # under-synchronized on HW), feat-major via PE transposes
                    ga_nt = qpool.tile([P, (QSL // P) * H], bf16, tag="ga_nt")
                    nc.gpsimd.dma_gather(
                        out_ap=ga_nt[:].rearrange("p (c e) -> p c e", e=H),
                        in_ap=a_tab,
                        idxs_ap=qa_sb[:, col : col + QSL // 16],
                        num_idxs=QSL,
                        num_idxs_reg=QSL,
                        elem_size=H,
                        single_packet=False,
                        queue_num=q_a,
                    )
                    gb_nt = qpool.tile([P, (QSL // P) * H], bf16, tag="gb_nt")
                    nc.gpsimd.dma_gather(
                        out_ap=gb_nt[:].rearrange("p (c e) -> p c e", e=H),
                        in_ap=b_tab,
                        idxs_ap=qb_sb[:, col : col + QSL // 16],
                        num_idxs=QSL,
                        num_idxs_reg=QSL,
                        elem_size=H,
                        single_packet=False,
                        queue_num=q_b,
                    )
                    za_ps = pp_t.tile([P, QSL], bf16, tag="za_ps")
                    zb_ps = pp_t.tile([P, QSL], bf16, tag="zb_ps")
                    for kk in range(QSL // P):
                        nc.tensor.transpose(
                            za_ps[:, kk * P : (kk + 1) * P],
                            ga_nt[:, kk * H : (kk + 1) * H], ident[:])
                        nc.tensor.transpose(
                            zb_ps[:, kk * P : (kk + 1) * P],
                            gb_nt[:, kk * H : (kk + 1) * H], ident[:])
                    zaT = qpool.tile([P, QSL], bf16, tag="zaT")
                    nc.scalar.copy(zaT[:], za_ps[:])
                    zbT = qpool.tile([P, QSL], bf16, tag="zbT")
                    nc.scalar.copy(zbT[:], zb_ps[:])
                    h_ps = pp_d.tile([P, QSL], f32, tag="h")
                    nc.tensor.matmul(out=h_ps[:], lhsT=e1[:], rhs=zaT[:], start=True, stop=False)
                    nc.tensor.matmul(out=h_ps[:], lhsT=e2[:], rhs=zbT[:], start=False, stop=True)
                    hT = qpool.tile([P, QSL], bf16, tag="hT")
                    nc.scalar.activation(hT[:], h_ps[:], AF.Relu, bias=bb[:])
                    l_ps = pp_d.tile([NCLS, QSL], f32, tag="l")
                    nc.tensor.matmul(out=l_ps[:], lhsT=dw2[:], rhs=hT[:], start=True, stop=True)
                    if sl % LB == 0:
                        lbuf = lpool.tile([NCLS, LB * QSL], f32, tag="lbuf")
                    o = (sl % LB) * QSL
                    nc.scalar.activation(lbuf[:, o : o + QSL], l_ps[:], AF.Identity, bias=db2[:])
                    if sl % LB == LB - 1 or sl == QS - 1:
                        base = (sl // LB) * LB * QSL
                        w = o + QSL
                        nc.sync.dma_start(logits_out[:, base : base + w], lbuf[:, :w])
                    sl += 1

    nc.compile()
    return nc


def kernel(**inputs):
    emb = np.asarray(inputs["emb"], np.float32)
    x = np.asarray(inputs["x"], np.int64)
    if not np.array_equal(x, np.arange(N_NODES)):
        emb = emb[x]

    newpos = _row_perm()

    def rowof(n):
        return newpos[n // P] * P + n % P

    sched, conv_pc, deg = _prep_conv(np.asarray(inputs["edge_index"], np.int64), rowof)
    dec, dec_pc, perms = _prep_decode(np.asarray(inputs["edge_label_index"], np.int64), rowof)

    nc = _build(sched, dec, sched["lo_cols"], sched["hi_cols"])

    W1 = np.asarray(inputs["W1"], np.float32)
    W2 = np.asarray(inputs["W2"], np.float32)
    b2 = np.asarray(inputs["b2"], np.float32)
    dW1 = np.asarray(inputs["dW1"], np.float32)
    db1 = np.asarray(inputs["db1"], np.float32)
    dis = (1.0 / np.sqrt(deg)).astype(np.float32)          # [NPAD]
    embW = np.zeros((NPAD, H), np.float32)
    embW[:N_NODES] = emb @ W1
    embpW = (embW * dis[:, None]).astype(BF16)             # [NPAD, H] bf16
    embp_tab = np.empty_like(embpW)                        # permuted row layout
    embp_tab[rowof(np.arange(NPAD))] = embpW
    dis_t = dis.reshape(NT, P).T                           # [P, NT]
    sdeg = np.sqrt(deg).astype(np.float32)                 # 1/dis

    E1 = np.ascontiguousarray(W2 @ dW1[:H])
    E2 = np.ascontiguousarray(W2 @ dW1[H:])
    bbias = (b2 @ dW1[:H] + b2 @ dW1[H:] + db1).astype(np.float32)

    in_maps = []
    for c in range(NCORES):
        t0 = c * TPC
        dis_s = np.ascontiguousarray(dis_t[:, t0 : t0 + TPC])
        self_rows = np.ascontiguousarray(embpW[t0 * P : (t0 + TPC) * P])
        sdeg_row = np.ascontiguousarray(sdeg[t0 * P : (t0 + TPC) * P].reshape(1, TPC * P))
        in_maps.append({
            "embp": embp_tab,
            "dis_s": dis_s,
            "sdeg_row": sdeg_row,
            "self_rows": self_rows,
            "b1": np.asarray(inputs["b1"], np.float32).reshape(1, H),
            "e1": E1,
            "e2": E2,
            "bbias": bbias.reshape(H, 1),
            "dw2": np.asarray(inputs["dW2"], np.float32),
            "db2": np.asarray(inputs["db2"], np.float32).reshape(NCLS, 1),
            "idx_lo": conv_pc[c]["idx_lo"],
            "idx_hi": conv_pc[c]["idx_hi"],
            "dstloc": conv_pc[c]["dstloc"],
            "qa": dec_pc[c]["qa"],
            "qb": dec_pc[c]["qb"],
        })

    res = bass_utils.run_bass_kernel_spmd(
        nc, in_maps, core_ids=list(range(NCORES)), trace=TRACE, **RUN_KWARGS
    )
    globals()["LAST_EXEC_NS"] = res.exec_time_ns
    globals()["LAST_RESULTS"] = res

    out = np.zeros((N_QUERY, NCLS), np.float32)
    for c in range(NCORES):
        lt = np.asarray(res.results[c]["logitsT"], np.float32).T  # [QPAD, NCLS]
        perm = perms[c]
        m = perm >= 0
        out[perm[m]] = lt[m]
    return out


if __name__ == "__main__":
    # lightweight self-check with a small random graph shape (full shapes)
    rng = np.random.default_rng(0)
    demo = {
        "x": np.arange(N_NODES, dtype=np.int64),
        "edge_index": rng.integers(0, N_NODES, (2, N_EDGES)),
        "edge_label_index": rng.integers(0, N_NODES, (2, N_QUERY)),
        "emb": rng.standard_normal((N_NODES, H), dtype=np.float32),
        "W1": rng.standard_normal((H, H), dtype=np.float32) * 0.08,
        "b1": np.zeros(H, np.float32),
        "W2": rng.standard_normal((H, H), dtype=np.float32) * 0.08,
        "b2": np.zeros(H, np.float32),
        "dW1": rng.standard_normal((2 * H, H), dtype=np.float32) * 0.06,
        "db1": rng.standard_normal(H, np.float32) * 0.06,
        "dW2": rng.standard_normal((H, NCLS), dtype=np.float32) * 0.08,
        "db2": rng.standard_normal(NCLS, np.float32) * 0.08,
    }
    out = kernel(**demo)
    print(out.shape, out.dtype, np.abs(out).mean())
